# revision 25
# baseline (speedup 1.0000x reference)
"""Grouped cross-attention Trainium2 kernel.

Problem: B=4, SQ=1024, SK=2048, D=1024, H=16 heads (HD=64), G=4 groups
(GD=256) grouped o_proj, key/query masks, softmax over keys.

Sharding: 8 cores = (batch b = c//2) x (half of heads s = c%2).
Each core computes attention for 8 heads (= 2 o_proj groups) of one batch
and produces out[b, :, s*512:(s+1)*512].

All matmuls bf16 (fp32 PSUM accumulation); masks folded away host-side:
  - Host gathers only unmasked keys/queries.  Key padding is handled by
    zeroing the ones-column of the augmented V beyond nk, so pad keys
    contribute exp(0)*0 = 0 to numerator and denominator (no mask bias).
    Padded query rows are discarded by the host scatter (no query mask).
  - Attention per (head-pair, key block): S matmuls (contraction 64) ->
    one PSUM bank each; ACT exp -> bf16; PV matmuls (contraction 128)
    accumulate [65, qn] (row 64 = softmax denominator via ones-column).
  - S matmuls are software-pipelined one key-block ahead of the PV
    matmuls so the in-order PE queue never stalls on the exp.
  - Normalization: stage denominator row to partition 0 (the custom-DVE
    reciprocal_approx_fast mis-executes on partition-offset inputs),
    approx-reciprocal, bf16 cast, PE outer-product broadcast (deferred
    into the next head-pair's S prefetch to hide its DVE dependency
    chain), DVE multiply into a shared [128, qn] tile per head pair.
  - o_proj: per group and 128-query tile: 2 matmuls (contraction 128 =
    2 heads) + bias add; group 0 first so the last head-pair's
    normalization overlaps group-0 matmuls.
  - Queries: one 512 chunk + narrow tail; the tail keeps all 9 key
    blocks of a head in ONE [128, nkc, qt] PSUM bank and does a single
    exp per head.
"""

import numpy as np
import ml_dtypes

import concourse.bass as bass
import concourse.mybir as mybir
import concourse.tile as tile
from concourse import bacc
from concourse.bass_utils import run_bass_kernel_spmd

f32 = mybir.dt.float32
bf16 = mybir.dt.bfloat16

B, SQ, SK, D, H, HD, G, GD = 4, 1024, 2048, 1024, 16, 64, 4, 256
NCORE = 8
DS = D // 2          # dims per core (8 heads)
HPC = 8              # heads per core
P = 128

TRACE = False        # test.py sets kernel.TRACE = True for profiling
LAST_RUN = {}        # test.py reads exec_time_ns etc. from here

_CACHE = {}


def _pad_up(n, m):
    return ((n + m - 1) // m) * m


def build_nc(sqp, skp):
    """Build the per-core Bass program for padded shapes [sqp, skp]."""
    nkc = skp // P
    qA = min(512, sqp)
    qB = sqp - qA
    assert 0 <= qB <= P

    nc = bacc.Bacc("TRN2", target_bir_lowering=False, debug=False,
                   num_devices=NCORE)

    qt_d = nc.dram_tensor("qt", [DS, sqp], bf16, kind="ExternalInput")
    kt_d = nc.dram_tensor("kt", [DS, skp], bf16, kind="ExternalInput")
    va_d = nc.dram_tensor("va", [skp, HPC * 2 * HD], bf16, kind="ExternalInput")
    wt_d = nc.dram_tensor("wt", [2, 2, P, GD], bf16, kind="ExternalInput")
    bb_d = nc.dram_tensor("bb", [P, DS], f32, kind="ExternalInput")
    out_d = nc.dram_tensor("out", [sqp, DS], f32, kind="ExternalOutput")

    with tile.TileContext(nc) as tc:
        with (
            tc.tile_pool(name="big", bufs=1) as big,
            tc.tile_pool(name="consts", bufs=1) as consts,
            tc.tile_pool(name="e_pool", bufs=4) as e_pool,
            tc.tile_pool(name="on_pool", bufs=8) as on_pool,
            tc.tile_pool(name="small", bufs=4) as small,
            tc.tile_pool(name="fo_pool", bufs=6) as fo_pool,
            tc.tile_pool(name="ps_s_pool", bufs=3, space="PSUM") as ps_s_pool,
            tc.tile_pool(name="ps_o_pool", bufs=3, space="PSUM") as ps_o_pool,
            tc.tile_pool(name="ps_x_pool", bufs=2, space="PSUM") as ps_x_pool,
        ):
            # ---- static loads (ordered by first use; kt0 per-block) ----
            kt_s, qt_s = [], []
            for j in range(4):
                t = big.tile([P, skp], bf16, tag=f"kt{j}", name=f"kt{j}")
                kt_s.append(t)
                t = big.tile([P, sqp], bf16, tag=f"qt{j}", name=f"qt{j}")
                qt_s.append(t)
            va_r = va_d.rearrange("(kc p) x -> kc p x", p=P)
            va_s = []
            for kc in range(nkc):
                t = big.tile([P, HPC, 2 * HD], bf16, tag=f"va{kc}",
                             name=f"va{kc}")
                va_s.append(t)
            # first needs: qt0 and kt0 blocks (hp=0's S), va blocks (PVs)
            nc.sync.dma_start(out=qt_s[0], in_=qt_d[0:P, :])
            for kc in range(nkc):
                nc.sync.dma_start(
                    out=kt_s[0][:, kc * P:(kc + 1) * P],
                    in_=kt_d[0:P, kc * P:(kc + 1) * P])
                nc.gpsimd.dma_start(
                    out=va_s[kc],
                    in_=va_r[kc].rearrange("p (h d) -> p h d", h=HPC))
            for j in range(1, 4):
                nc.scalar.dma_start(out=qt_s[j],
                                    in_=qt_d[j * P:(j + 1) * P, :])
                nc.scalar.dma_start(out=kt_s[j],
                                    in_=kt_d[j * P:(j + 1) * P, :])
            wt_s = []
            for g in range(2):
                for p in range(2):
                    t = consts.tile([P, GD], bf16, tag=f"wt{g}{p}")
                    nc.sync.dma_start(out=t, in_=wt_d[g, p])
                    wt_s.append(t)
            bb_s = consts.tile([P, DS], f32)
            nc.sync.dma_start(out=bb_s, in_=bb_d[:, :])
            Exp = mybir.ActivationFunctionType.Exp

            def norm_head(ps_o, on_view, qn):
                """1/denominator broadcast to 64 partitions, then scale.
                No PE involvement: the broadcast rides on GpSimd/DMA."""
                den = small.tile([1, qn], f32, tag="den")
                nc.vector.tensor_copy(den[:, :], ps_o[HD:HD + 1, :])
                recip = small.tile([1, qn], f32, tag="recip")
                nc.vector.reciprocal_approx_fast(recip[:, :], den[:, :])
                bcast = small.tile([HD, qn], f32, tag="bcast")
                nc.gpsimd.partition_broadcast(bcast[:, :], recip[:, :])
                nc.vector.tensor_mul(on_view, ps_o[0:HD, :], bcast[:, :])

            # ---- chunk A: 512-wide, S one key-block ahead of PV ----
            on_A, on_B = [], []
            for hp in range(4):
                h0, h1 = 2 * hp, 2 * hp + 1
                ps_o0 = ps_o_pool.tile([P, qA], f32, tag="ps_o")
                ps_o1 = ps_o_pool.tile([P, qA], f32, tag="ps_o")
                ss, es = {}, {}

                def do_S(kc, hp=hp, ss=ss):
                    pair = []
                    for off in (0, HD):
                        t = ps_s_pool.tile([P, qA], f32, tag="ps_s")
                        nc.tensor.matmul(
                            t[:, :],
                            kt_s[hp][off:off + HD, kc * P:(kc + 1) * P],
                            qt_s[hp][off:off + HD, 0:qA],
                            start=True, stop=True)
                        pair.append(t)
                    ss[kc] = pair

                def do_exp(kc, ss=ss, es=es):
                    pair = []
                    for p in range(2):
                        e = e_pool.tile([P, qA], bf16, tag="e")
                        nc.scalar.activation(e[:, :], ss[kc][p][:, :],
                                             Exp, scale=0.125)
                        pair.append(e)
                    es[kc] = pair

                do_S(0)
                do_exp(0)
                do_S(1)
                for kc in range(nkc):
                    if kc + 1 < nkc:
                        do_exp(kc + 1)
                    if kc + 2 < nkc:
                        do_S(kc + 2)
                    for p, (h, ps_o) in enumerate(((h0, ps_o0), (h1, ps_o1))):
                        nc.tensor.matmul(
                            ps_o[:, :], va_s[kc][:, h, :], es[kc][p][:, :],
                            start=(kc == 0), stop=(kc == nkc - 1))
                    del ss[kc]

                on2 = on_pool.tile([P, qA], bf16, tag="on")
                norm_head(ps_o0, on2[0:HD, :], qA)
                norm_head(ps_o1, on2[HD:2 * HD, :], qA)
                on_A.append(on2)

            # ---- o_proj helper: emitted per chunk so chunk A's
            # projection overlaps chunk B's attention ----
            def o_proj(tiles):
                fo_s = []
                for _ in tiles:
                    fo = fo_pool.tile([P, DS], f32, tag="fo")
                    fo_s.append(fo)
                for g in range(2):
                    for ti_idx, (q0, t_i, tw, on_src) in enumerate(tiles):
                        ps_out = ps_x_pool.tile([P, GD], f32, tag="ps_x")
                        for p in range(2):
                            nc.tensor.matmul(
                                ps_out[0:tw, :],
                                on_src[2 * g + p][:, t_i * P:t_i * P + tw],
                                wt_s[2 * g + p][:, :],
                                start=(p == 0), stop=(p == 1))
                        fo = fo_s[ti_idx]
                        nc.vector.tensor_add(
                            fo[0:tw, g * GD:(g + 1) * GD], ps_out[0:tw, :],
                            bb_s[0:tw, g * GD:(g + 1) * GD])
                        nc.sync.dma_start(
                            out=out_d[q0 + t_i * P: q0 + t_i * P + tw,
                                      g * GD:(g + 1) * GD],
                            in_=fo[0:tw, g * GD:(g + 1) * GD])

            o_proj([(0, t_i, min(P, qA - t_i * P), on_A)
                    for t_i in range((qA + P - 1) // P)])

            # ---- chunk B (tail): all key blocks in one bank per head ----
            if qB:
                for hp in range(4):
                    h0, h1 = 2 * hp, 2 * hp + 1
                    ps_o0 = ps_o_pool.tile([P, qB], f32, tag="ps_o")
                    ps_o1 = ps_o_pool.tile([P, qB], f32, tag="ps_o")
                    ts, te = [], []
                    for p, off in ((0, 0), (1, HD)):
                        t = ps_s_pool.tile([P, nkc, qB], f32, tag="ps_s")
                        for kc in range(nkc):
                            nc.tensor.matmul(
                                t[:, kc, :],
                                kt_s[hp][off:off + HD, kc * P:(kc + 1) * P],
                                qt_s[hp][off:off + HD, qA:qA + qB],
                                start=True, stop=True)
                        ts.append(t)
                    for p in range(2):
                        e = e_pool.tile([P, nkc, qB], bf16, tag="e")
                        nc.scalar.activation(e[:, :, :], ts[p][:, :, :],
                                             Exp, scale=0.125)
                        te.append(e)
                    for kc in range(nkc):
                        for p, (h, ps_o) in enumerate(((h0, ps_o0),
                                                       (h1, ps_o1))):
                            nc.tensor.matmul(
                                ps_o[:, :], va_s[kc][:, h, :],
                                te[p][:, kc, :],
                                start=(kc == 0), stop=(kc == nkc - 1))
                    on2 = on_pool.tile([P, qB], bf16, tag="on")
                    norm_head(ps_o0, on2[0:HD, :], qB)
                    norm_head(ps_o1, on2[HD:2 * HD, :], qB)
                    on_B.append(on2)
                o_proj([(qA, 0, qB, on_B)])


    nc.compile()
    return nc


def _prep_core_inputs(c, sqp, skp, q_idx, k_idx, query, key, value,
                      o_weight, o_bias):
    """Build the per-core input map. q_idx/k_idx are the compressed row
    indices per batch."""
    b, s = c // 2, c % 2
    dsl = slice(s * DS, (s + 1) * DS)

    qi, ki = q_idx[b], k_idx[b]
    nq, nk = len(qi), len(ki)

    qsl = query[b][qi][:, dsl]                       # [nq, DS]
    qt = np.zeros((DS, sqp), np.float32)
    qt[:, :nq] = qsl.T
    ksl = key[b][ki][:, dsl]
    kt = np.zeros((DS, skp), np.float32)
    kt[:, :nk] = ksl.T
    va = np.zeros((skp, HPC, 2 * HD), np.float32)
    va[:nk, :, :HD] = value[b][ki][:, dsl].reshape(nk, HPC, HD)
    va[:nk, :, HD] = 1.0                             # pad keys stay 0 -> masked
    va = va.reshape(skp, HPC * 2 * HD)

    # o_weight[2s+g].T is [in, out]; split 256 contraction rows into two
    # blocks of 128 (head pairs)
    wt = np.stack([o_weight[2 * s + g].T.reshape(2, P, GD) for g in range(2)])
    bb = np.broadcast_to(o_bias[dsl].astype(np.float32), (P, DS))
    b16 = ml_dtypes.bfloat16
    return {"qt": np.ascontiguousarray(qt.astype(b16)),
            "kt": np.ascontiguousarray(kt.astype(b16)),
            "va": np.ascontiguousarray(va.astype(b16)),
            "wt": np.ascontiguousarray(wt.astype(b16)),
            "bb": np.ascontiguousarray(bb)}


def kernel(query, key, value, key_mask, query_mask, o_weight, o_bias):
    query = np.asarray(query, np.float32)
    key = np.asarray(key, np.float32)
    value = np.asarray(value, np.float32)
    key_mask = np.asarray(key_mask)
    query_mask = np.asarray(query_mask)
    o_weight = np.asarray(o_weight, np.float32)
    o_bias = np.asarray(o_bias, np.float32)

    k_idx = [np.nonzero(key_mask[b, :, 0])[0] for b in range(B)]
    q_idx = [np.nonzero(query_mask[b, :, 0])[0] for b in range(B)]
    skp = max(P, _pad_up(max(len(i) for i in k_idx), P))
    sqp = max(32, _pad_up(max(len(i) for i in q_idx), 32))
    if sqp > 512 + P:                 # tail must fit one query tile
        sqp = _pad_up(sqp, P)

    if (sqp, skp) not in _CACHE:
        _CACHE[(sqp, skp)] = build_nc(sqp, skp)
    nc = _CACHE[(sqp, skp)]

    in_maps = [
        _prep_core_inputs(c, sqp, skp, q_idx, k_idx, query, key, value,
                          o_weight, o_bias)
        for c in range(NCORE)
    ]
    res = run_bass_kernel_spmd(nc, in_maps, core_ids=list(range(NCORE)),
                               trace=TRACE)
    LAST_RUN["exec_time_ns"] = res.exec_time_ns
    LAST_RUN["profile_json"] = res.profile_json
    LAST_RUN["results"] = res

    out = np.empty((B, SQ, D), np.float32)
    for c in range(NCORE):
        b, s = c // 2, c % 2
        core_out = np.asarray(res.results[c]["out"], np.float32)
        qi = q_idx[b]
        out[b, :, s * DS:(s + 1) * DS] = o_bias[s * DS:(s + 1) * DS]
        out[b, qi, s * DS:(s + 1) * DS] = core_out[:len(qi)]
    return out
